# revision 27
# baseline (speedup 1.0000x reference)
"""BertEmbeddings (word lookup + header mean-pool scatter + pos/type/match
embeddings + TF-style LayerNorm) as a Bass/Tile kernel on 8 trn2 NeuronCores.

Sharding: data-parallel over batch (4 rows/core); embedding tables replicated.

Per-core device pipeline (v2 — engine-balanced):
  - word rows arrive via an ACCUMULATING indirect DMA gather on top of a
    pos_emb prefill (SBUF->SBUF DMA), so words+pos costs zero engine time
  - header rows (slot order, pad slots -> zero row) gathered with dma_gather,
    mean-pooled with a free-dim add tree; pooled slot rows are converted to
    REPLACEMENT deltas (pooled - word_at_target) so the scatter needs no mask
  - small tables (tok/match/type concat [19,768]) + scatter deltas are added
    in one PSUM accumulation via bf16 one-hot matmuls (host-built one-hots)
  - emb = (words+pos) + PSUM in one tensor_tensor_reduce that also emits the
    row sum; sum of squares comes from an ACT Square pass (accum_out); the
    LN apply (x-u)*rstd runs on ACT as Identity(scale=rstd, bias=-u*rstd)

All data-dependent arithmetic on embedding VALUES runs on device; the host
only reformats index tensors (gather index layouts, zero-row padding, slot
permutation by col_idx, one-hot/multi-hot index encodings).
"""

import numpy as np

B, S, H = 32, 512, 768
VOCAB = 30522
NCORES = 8
BPC = B // NCORES            # batch rows per core
T = BPC * S                  # tokens per core
NBLK = T // 128              # 128-token blocks per core
C, L = 32, 16                # columns, max header len
NSLOT = BPC * C              # 128 slots per core
ZROW = VOCAB                 # zero row in augmented word table
WROWS = VOCAB + 1
NV = 19                      # 2 + 11 + 6 small-table rows
EPS = 1e-12

_NC_CACHE = {}

import os as _os

# bisect/tuning knobs (read at build time)
OPT_SCRATCH = int(_os.environ.get("K_SCRATCH", "16384"))
OPT_ACC_GATHER = _os.environ.get("K_ACC", "1") == "1"   # accumulate words onto pos prefill
OPT_INDIRECT = _os.environ.get("K_IND", "1") == "1"     # indirect dma for word gathers
OPT_ACT_STATS = _os.environ.get("K_ACTSTATS", "1") == "1"  # sumsq via ACT accum
OPT_ACT_APPLY = _os.environ.get("K_ACTAPPLY", "1") == "1"  # LN apply on ACT
# tensor_tensor_reduce crashes the exec unit on HW (2026-08) — keep off
OPT_TTR = _os.environ.get("K_TTR", "0") == "1"
OPT_PE_POS = _os.environ.get("K_PEPOS", "1") == "1"  # pos rows via identity matmul


def _build_nc(skip_affine: bool):
    from contextlib import ExitStack

    import concourse.bacc as bacc
    import concourse.tile as tile
    from concourse import mybir

    BF16 = mybir.dt.bfloat16
    I16 = mybir.dt.int16
    I32 = mybir.dt.int32
    F32 = mybir.dt.float32

    nc = bacc.Bacc(
        "TRN2", target_bir_lowering=False, debug=False,
        dynamic_dma_scratch_size=OPT_SCRATCH,
    )
    t = {}

    def inp(name, shape, dt=F32):
        t[name] = nc.dram_tensor(name, shape, dt, kind="ExternalInput").ap()

    inp("word_aug", [WROWS, H])
    inp("pos_emb", [S, H])
    inp("small3", [2 * NV, H], BF16)   # [hi; lo] error-compensated split
    if not skip_affine:
        inp("lnw", [1, H])
        inp("lnb", [1, H])
    inp("mh", [2 * NV, T], BF16)   # multi-hot small-table encoding, stacked x2
    inp("oh", [128, T], BF16)      # one-hot slot->target-column scatter matrix
    inp("hl", [128, 1])            # header_len per slot (selected by col_idx)
    inp("widx", [128, NBLK], I32)  # word row per (p, j) token
    inp("wtidx", [128, 1], I32)    # word row at each slot's target token
    inp("hidx", [128, T // 16], I16)  # header gather rows, wrap16 layout
    inp("widx16", [128, T // 16], I16)   # word rows, wrap16 (dma_gather path)
    inp("wtidx16", [128, NSLOT // 16], I16)  # target word rows, wrap16
    inp("eye", [128, 128], BF16)         # identity for the pos matmul
    inp("poshi", [128, BPC * H], BF16)   # pos rows (p,j)-layout, hi part
    inp("poslo", [128, BPC * H], BF16)   # pos rows (p,j)-layout, lo part
    out = nc.dram_tensor("out", [BPC, S, H], F32, kind="ExternalOutput").ap()

    with tile.TileContext(nc) as tc, ExitStack() as ctx:
        _body(ctx, tc, t, out, skip_affine, mybir)
    nc.compile()
    return nc


def _body(ctx, tc, t, out, skip_affine, mybir):
    import concourse.bass as bass

    nc = tc.nc
    F32 = mybir.dt.float32
    BF16 = mybir.dt.bfloat16
    I16 = mybir.dt.int16
    I32 = mybir.dt.int32
    MUL = mybir.AluOpType.mult
    ADD = mybir.AluOpType.add
    AF = mybir.ActivationFunctionType

    const = ctx.enter_context(tc.tile_pool(name="const", bufs=1))
    setup = ctx.enter_context(tc.tile_pool(name="setup", bufs=1))
    hpool = ctx.enter_context(tc.tile_pool(name="hdr", bufs=3))
    h2pool = ctx.enter_context(tc.tile_pool(name="h2", bufs=2))
    h1pool = ctx.enter_context(tc.tile_pool(name="h1", bufs=2))
    wpool = ctx.enter_context(tc.tile_pool(name="wrd", bufs=3))
    epool = ctx.enter_context(tc.tile_pool(name="emb", bufs=3))
    opool = ctx.enter_context(tc.tile_pool(name="outp", bufs=3))
    qpool = ctx.enter_context(tc.tile_pool(name="sq", bufs=2))
    spool = ctx.enter_context(tc.tile_pool(name="stat", bufs=8))
    psum = ctx.enter_context(tc.tile_pool(name="ps", bufs=3, space="PSUM"))

    # ---------------- constants / index tiles ----------------
    s_widx = const.tile([128, NBLK], I32)
    nc.sync.dma_start(s_widx[:], t["widx"])
    s_wtidx = const.tile([128, 1], I32)
    nc.sync.dma_start(s_wtidx[:], t["wtidx"])
    s_hidx = const.tile([128, T // 16], I16)
    nc.sync.dma_start(s_hidx[:], t["hidx"])

    s_small = const.tile([2 * NV, H], BF16)
    nc.sync.dma_start(s_small[:], t["small3"])
    s_mh = const.tile([2 * NV, T], BF16)
    nc.sync.dma_start(s_mh[:], t["mh"])
    s_oh = const.tile([128, T], BF16)
    nc.sync.dma_start(s_oh[:], t["oh"])

    if OPT_PE_POS:
        s_eye = const.tile([128, 128], BF16)
        nc.sync.dma_start(s_eye[:], t["eye"])
        s_poshi = const.tile([128, BPC, H], BF16)
        nc.sync.dma_start(
            s_poshi[:], t["poshi"].rearrange("p (j h) -> p j h", h=H)
        )
        s_poslo = const.tile([128, BPC, H], BF16)
        nc.sync.dma_start(
            s_poslo[:], t["poslo"].rearrange("p (j h) -> p j h", h=H)
        )
        s_pos = None
    else:
        s_pos = const.tile([128, BPC, H], F32)
        nc.sync.dma_start(
            s_pos[:], t["pos_emb"].rearrange("(j p) h -> p j h", p=128)
        )

    s_eps = const.tile([128, 1], F32)
    nc.vector.memset(s_eps[:], EPS)

    if not skip_affine:
        s_lnw = const.tile([128, H], F32)
        nc.gpsimd.dma_start(s_lnw[:], t["lnw"].partition_broadcast(128))
        s_lnb = const.tile([128, H], F32)
        nc.gpsimd.dma_start(s_lnb[:], t["lnb"].partition_broadcast(128))

    s_hl = const.tile([128, 1], F32)
    nc.sync.dma_start(s_hl[:], t["hl"])
    s_recip = const.tile([128, 1], F32)
    nc.vector.tensor_scalar_max(s_recip[:], s_hl[:], 1.0)
    nc.vector.reciprocal(s_recip[:], s_recip[:])

    # ---------------- header gather + pooling ----------------
    # gather order i2 = l*128 + slot  ->  hch[:, m, :] holds l = 4*lc + m
    hacc = setup.tile([128, H], F32)
    for lc in range(4):
        hch = hpool.tile([128, 4, H], F32)
        nc.gpsimd.dma_gather(
            hch[:], t["word_aug"], s_hidx[:, 32 * lc : 32 * (lc + 1)], 512, 512, H
        )
        h2 = h2pool.tile([128, 2, H], F32)
        nc.vector.tensor_add(h2[:], hch[:, 0:2, :], hch[:, 2:4, :])
        if lc == 0:
            nc.vector.tensor_add(hacc[:], h2[:, 0, :], h2[:, 1, :])
        else:
            h1 = h1pool.tile([128, H], F32)
            nc.vector.tensor_add(h1[:], h2[:, 0, :], h2[:, 1, :])
            nc.vector.tensor_add(hacc[:], hacc[:], h1[:])

    # word rows at the scatter target tokens (for replacement deltas)
    wtgt = setup.tile([128, H], F32)
    if OPT_INDIRECT:
        nc.gpsimd.indirect_dma_start(
            wtgt[:], None, t["word_aug"],
            bass.IndirectOffsetOnAxis(ap=s_wtidx[:, 0:1], axis=0),
        )
    else:
        s_wtidx16 = const.tile([128, NSLOT // 16], I16)
        nc.sync.dma_start(s_wtidx16[:], t["wtidx16"])
        wtgt3 = wtgt[:].rearrange("p (o h) -> p o h", o=1)
        nc.gpsimd.dma_gather(
            wtgt3, t["word_aug"], s_wtidx16[:], NSLOT, NSLOT, H
        )
    # hadj = pooled - word_at_target (bf16 rhs for the scatter matmul);
    # invalid slots contribute nothing (their one-hot column is all zero)
    d = setup.tile([128, H], F32)
    nc.vector.tensor_scalar_mul(d[:], hacc[:], s_recip[:])
    nc.vector.tensor_sub(d[:], d[:], wtgt[:])
    hadj_hi = setup.tile([128, H], BF16)
    nc.vector.tensor_copy(hadj_hi[:], d[:])
    hadj_lo = setup.tile([128, H], BF16)
    nc.vector.tensor_sub(hadj_lo[:], d[:], hadj_hi[:])

    s_widx16 = None
    if OPT_PE_POS or not OPT_INDIRECT:
        s_widx16 = const.tile([128, T // 16], I16)
        nc.sync.dma_start(s_widx16[:], t["widx16"])

    # ---------------- token blocks ----------------
    inv_h = 1.0 / H
    for ch in range(BPC):
        wch = wpool.tile([128, 4, H], F32)
        if OPT_PE_POS:
            nc.gpsimd.dma_gather(
                wch[:], t["word_aug"], s_widx16[:, 32 * ch : 32 * (ch + 1)],
                512, 512, H,
            )
        elif OPT_ACC_GATHER:
            # prefill with positional rows, then accumulate gathered words
            # (one [P,1]-indexed indirect gather per 128-token block)
            nc.sync.dma_start(wch[:], s_pos[:])
            for jj in range(4):
                nc.gpsimd.indirect_dma_start(
                    wch[:, jj, :], None, t["word_aug"],
                    bass.IndirectOffsetOnAxis(
                        ap=s_widx[:, 4 * ch + jj : 4 * ch + jj + 1], axis=0
                    ),
                    compute_op=ADD,
                )
        elif OPT_INDIRECT:
            nc.gpsimd.indirect_dma_start(
                wch[:], None, t["word_aug"],
                bass.IndirectOffsetOnAxis(
                    ap=s_widx[:, 4 * ch : 4 * (ch + 1)], axis=0
                ),
            )
        else:
            nc.gpsimd.dma_gather(
                wch[:], t["word_aug"], s_widx16[:, 32 * ch : 32 * (ch + 1)],
                512, 512, H,
            )
        for jj in range(4):
            j = ch * 4 + jj
            ps = psum.tile([128, H], F32)
            lhs_mh = s_mh[:, j * 128 : (j + 1) * 128]
            lhs_oh = s_oh[:, j * 128 : (j + 1) * 128]
            for lo, hi in ((0, 512), (512, H)):
                nc.tensor.matmul(
                    ps[:, lo:hi], lhs_mh, s_small[:, lo:hi],
                    start=True, stop=False,
                )
                nc.tensor.matmul(
                    ps[:, lo:hi], lhs_oh, hadj_hi[:, lo:hi],
                    start=False, stop=False,
                )
                nc.tensor.matmul(
                    ps[:, lo:hi], lhs_oh, hadj_lo[:, lo:hi],
                    start=False, stop=not OPT_PE_POS,
                )
                if OPT_PE_POS:
                    nc.tensor.matmul(
                        ps[:, lo:hi], s_eye[:], s_poshi[:, jj, lo:hi],
                        start=False, stop=False,
                    )
                    nc.tensor.matmul(
                        ps[:, lo:hi], s_eye[:], s_poslo[:, jj, lo:hi],
                        start=False, stop=True,
                    )

            if OPT_PE_POS or OPT_ACC_GATHER:
                base = wch[:, jj, :]
            else:
                basetile = epool.tile([128, H], F32, tag="base")
                nc.vector.tensor_add(basetile[:], wch[:, jj, :], s_pos[:, jj, :])
                base = basetile[:]

            emb = epool.tile([128, H], F32)
            rsum = spool.tile([128, 1], F32)
            if OPT_TTR:
                # emb = (words+pos) + ps, and the row-sum, in one DVE pass
                nc.vector.tensor_tensor_reduce(
                    emb[:], base, ps[:], 1.0, 0.0, ADD, ADD, rsum[:]
                )
            else:
                nc.vector.tensor_add(emb[:], base, ps[:])

            u = spool.tile([128, 1], F32)
            var = spool.tile([128, 1], F32)
            if OPT_ACT_STATS:
                if not OPT_TTR:
                    # row sum on ACT (output write is a throwaway)
                    cp = qpool.tile([128, H], F32, tag="cp")
                    nc.scalar.activation(
                        cp[:], emb[:], AF.Copy, accum_out=rsum[:]
                    )
                # sum of squares on ACT
                sq = qpool.tile([128, H], F32)
                rsumsq = spool.tile([128, 1], F32)
                nc.scalar.activation(
                    sq[:], emb[:], AF.Square, accum_out=rsumsq[:]
                )
                # u, var  (tiny per-partition ops)
                nc.vector.tensor_scalar_mul(u[:], rsum[:], inv_h)
                mu2 = spool.tile([128, 1], F32)
                nc.vector.tensor_scalar(
                    mu2[:], u[:], u[:], -1.0, op0=MUL, op1=MUL
                )
                nc.vector.tensor_scalar(
                    var[:], rsumsq[:], inv_h, mu2[:], op0=MUL, op1=ADD
                )
            else:
                stats = spool.tile([128, 2, 6], F32)
                for g in range(2):
                    nc.vector.bn_stats(
                        stats[:, g, :], emb[:, g * 384 : (g + 1) * 384]
                    )
                mv = spool.tile([128, 2], F32)
                nc.vector.bn_aggr(mv[:], stats[:])
                nc.vector.tensor_copy(u[:], mv[:, 0:1])
                nc.vector.tensor_copy(var[:], mv[:, 1:2])

            rstd = spool.tile([128, 1], F32)
            nc.scalar.activation(
                rstd[:], var[:], AF.Sqrt, bias=s_eps[:], scale=1.0
            )
            nc.vector.reciprocal(rstd[:], rstd[:])

            o = opool.tile([128, H], F32)
            if OPT_ACT_APPLY:
                nub = spool.tile([128, 1], F32)
                nc.vector.tensor_scalar(
                    nub[:], u[:], rstd[:], -1.0, op0=MUL, op1=MUL
                )
                # LN apply on ACT: out = rstd*emb - u*rstd
                nc.scalar.activation(
                    o[:], emb[:], AF.Identity, bias=nub[:], scale=rstd[:]
                )
            else:
                nc.vector.tensor_scalar(
                    o[:], emb[:], u[:], rstd[:],
                    op0=mybir.AluOpType.subtract, op1=MUL,
                )
            if not skip_affine:
                nc.vector.tensor_mul(o[:], o[:], s_lnw[:])
                nc.vector.tensor_add(o[:], o[:], s_lnb[:])

            nc.sync.dma_start(out[ch, jj * 128 : (jj + 1) * 128, :], o[:])


def _wrap16(flat):
    w = flat.reshape(-1, 16).T.astype(np.int16)
    return np.tile(w, (8, 1))


def _prep_core(core, iid, hdr, tt, mt, ti, cpos, cidx, hlen):
    import ml_dtypes

    b0 = core * BPC
    sl = slice(b0, b0 + BPC)
    iids = iid[sl]

    # word gather rows in (p, ch*4+j) layout for the indirect gathers
    widx = np.ascontiguousarray(
        iids.reshape(BPC * 4, 128).T.astype(np.int32)
    )  # [128, NBLK] ; widx[p, j] = token j*128+p

    bb = np.arange(BPC)[:, None]
    sel_hdr = hdr[sl][bb, cidx[sl]]                      # [BPC, C, L]
    sel_len = hlen[sl][bb, cidx[sl]]                     # [BPC, C]
    maskl = np.arange(L)[None, None, :] < sel_len[:, :, None]
    hvals = np.where(maskl, sel_hdr, ZROW)               # [BPC, C, L]
    hflat = hvals.reshape(NSLOT, L).T.reshape(-1)        # i2 = l*128 + slot
    hidx = _wrap16(hflat)

    # word rows at each slot's target position
    wtidx = iids[bb, cpos[sl]].reshape(NSLOT, 1).astype(np.int32)

    # one-hot scatter matrix [128 slots, T] (bf16): column = local target token
    tgt = np.where(
        sel_len.reshape(-1) > 0, (bb * S + cpos[sl]).reshape(-1), -1
    )
    oh = np.zeros((NSLOT, T), dtype=ml_dtypes.bfloat16)
    valid = tgt >= 0
    oh[np.arange(NSLOT)[valid], tgt[valid]] = 1

    # multi-hot small-table encoding [19, T], stacked twice for the
    # hi/lo error-compensated small-table matmul
    mh1 = np.zeros((NV, T), dtype=ml_dtypes.bfloat16)
    ar = np.arange(T)
    mh1[tt[sl].reshape(-1), ar] = 1
    mh1[2 + mt[sl].reshape(-1), ar] += 1
    mh1[13 + ti[sl].reshape(-1), ar] += 1
    mh = np.concatenate([mh1, mh1], axis=0)

    hl = sel_len.reshape(NSLOT, 1).astype(np.float32)
    widx16 = _wrap16(iids.reshape(-1))
    wtidx16 = _wrap16(wtidx.reshape(-1))
    return widx, wtidx, hidx, oh, mh, hl, widx16, wtidx16


def make_in_maps(inputs):
    import ml_dtypes

    inp = {k: np.asarray(v) for k, v in inputs.items()}
    word = np.ascontiguousarray(inp["word_emb"], dtype=np.float32)
    word_aug = np.concatenate([word, np.zeros((1, H), np.float32)], axis=0)
    small3_f32 = np.concatenate(
        [inp["tok_type_emb"], inp["match_emb"], inp["type_emb"]], axis=0
    ).astype(np.float32)
    small_hi = small3_f32.astype(ml_dtypes.bfloat16)
    small_lo = (small3_f32 - small_hi.astype(np.float32)).astype(
        ml_dtypes.bfloat16
    )
    small3 = np.concatenate([small_hi, small_lo], axis=0)
    pos = np.ascontiguousarray(inp["pos_emb"], dtype=np.float32)
    lnw = np.ascontiguousarray(inp["ln_w"], dtype=np.float32).reshape(1, H)
    lnb = np.ascontiguousarray(inp["ln_b"], dtype=np.float32).reshape(1, H)
    skip_affine = bool(np.all(lnw == 1.0) and np.all(lnb == 0.0))

    iid = inp["input_ids"].astype(np.int64)
    hdr = inp["header_ids"].astype(np.int64)
    tt = inp["token_type_ids"].astype(np.int64)
    mt = inp["match_type_ids"].astype(np.int64)
    ti = inp["type_idx"].astype(np.int64)
    cpos = inp["col_pos"].astype(np.int64)
    cidx = inp["col_idx"].astype(np.int64)
    hlen = inp["header_len"].astype(np.int64)

    # pos rows in (p, j)-block layout, split hi/lo for exact bf16 matmul adds
    posp = np.ascontiguousarray(
        pos.reshape(4, 128, H).transpose(1, 0, 2).reshape(128, 4 * H)
    )
    poshi = posp.astype(ml_dtypes.bfloat16)
    poslo = (posp - poshi.astype(np.float32)).astype(ml_dtypes.bfloat16)
    eye = np.eye(128, dtype=ml_dtypes.bfloat16)

    in_maps = []
    for core in range(NCORES):
        widx, wtidx, hidx, oh, mh, hl, widx16, wtidx16 = _prep_core(
            core, iid, hdr, tt, mt, ti, cpos, cidx, hlen
        )
        m = dict(
            word_aug=word_aug, pos_emb=pos, small3=small3,
            mh=mh, oh=oh, hl=hl, widx=widx, wtidx=wtidx, hidx=hidx,
            widx16=widx16, wtidx16=wtidx16,
            eye=eye, poshi=poshi, poslo=poslo,
        )
        if not skip_affine:
            m["lnw"] = lnw
            m["lnb"] = lnb
        in_maps.append(m)
    return in_maps, skip_affine


def get_nc(skip_affine):
    if skip_affine not in _NC_CACHE:
        _NC_CACHE[skip_affine] = _build_nc(skip_affine)
    return _NC_CACHE[skip_affine]


def run_hw(inputs, trace=False, trace_cores=None):
    """Returns (out [B,S,H] f32, BassKernelResults)."""
    from concourse.bass_utils import run_bass_kernel_spmd

    in_maps, skip_affine = make_in_maps(inputs)
    nc = get_nc(skip_affine)
    res = run_bass_kernel_spmd(
        nc, in_maps, core_ids=list(range(NCORES)), trace=trace,
        trace_cores=trace_cores,
    )
    out = np.concatenate([res.results[c]["out"] for c in range(NCORES)], axis=0)
    return out, res


def kernel(**inputs) -> np.ndarray:
    out, _ = run_hw(inputs, trace=False)
    return out


# revision 32
# speedup vs baseline: 1.1219x; 1.1219x over previous
"""BertEmbeddings (word lookup + header mean-pool scatter + pos/type/match
embeddings + TF-style LayerNorm) as a Bass/Tile kernel on 8 trn2 NeuronCores.

Sharding: data-parallel over batch (4 rows/core); embedding tables replicated.

Per-core device pipeline (v2 — engine-balanced):
  - word rows arrive via an ACCUMULATING indirect DMA gather on top of a
    pos_emb prefill (SBUF->SBUF DMA), so words+pos costs zero engine time
  - header rows (slot order, pad slots -> zero row) gathered with dma_gather,
    mean-pooled with a free-dim add tree; pooled slot rows are converted to
    REPLACEMENT deltas (pooled - word_at_target) so the scatter needs no mask
  - small tables (tok/match/type concat [19,768]) + scatter deltas are added
    in one PSUM accumulation via bf16 one-hot matmuls (host-built one-hots)
  - emb = (words+pos) + PSUM in one tensor_tensor_reduce that also emits the
    row sum; sum of squares comes from an ACT Square pass (accum_out); the
    LN apply (x-u)*rstd runs on ACT as Identity(scale=rstd, bias=-u*rstd)

All data-dependent arithmetic on embedding VALUES runs on device; the host
only reformats index tensors (gather index layouts, zero-row padding, slot
permutation by col_idx, one-hot/multi-hot index encodings).
"""

import numpy as np

B, S, H = 32, 512, 768
VOCAB = 30522
NCORES = 8
BPC = B // NCORES            # batch rows per core
T = BPC * S                  # tokens per core
NBLK = T // 128              # 128-token blocks per core
C, L = 32, 16                # columns, max header len
NSLOT = BPC * C              # 128 slots per core
ZROW = VOCAB                 # zero row in augmented word table
WROWS = VOCAB + 1
NV = 19                      # 2 + 11 + 6 small-table rows
EPS = 1e-12

_NC_CACHE = {}

import os as _os

# bisect/tuning knobs (read at build time)
OPT_SCRATCH = int(_os.environ.get("K_SCRATCH", "16384"))
OPT_ACC_GATHER = _os.environ.get("K_ACC", "1") == "1"   # accumulate words onto pos prefill
OPT_INDIRECT = _os.environ.get("K_IND", "1") == "1"     # indirect dma for word gathers
OPT_ACT_STATS = _os.environ.get("K_ACTSTATS", "1") == "1"  # sumsq via ACT accum
OPT_ACT_APPLY = _os.environ.get("K_ACTAPPLY", "1") == "1"  # LN apply on ACT
# tensor_tensor_reduce crashes the exec unit on HW (2026-08) — keep off
OPT_TTR = _os.environ.get("K_TTR", "0") == "1"
OPT_PE_POS = _os.environ.get("K_PEPOS", "1") == "1"  # pos rows via identity matmul


def _build_nc(skip_affine: bool):
    from contextlib import ExitStack

    import concourse.bacc as bacc
    import concourse.tile as tile
    from concourse import mybir

    BF16 = mybir.dt.bfloat16
    I16 = mybir.dt.int16
    I32 = mybir.dt.int32
    F32 = mybir.dt.float32

    nc = bacc.Bacc(
        "TRN2", target_bir_lowering=False, debug=False,
        dynamic_dma_scratch_size=OPT_SCRATCH,
    )
    t = {}

    def inp(name, shape, dt=F32):
        t[name] = nc.dram_tensor(name, shape, dt, kind="ExternalInput").ap()

    inp("word_aug", [WROWS, H])
    inp("pos_emb", [S, H])
    inp("small3", [2 * NV, H], BF16)   # [hi; lo] error-compensated split
    if not skip_affine:
        inp("lnw", [1, H])
        inp("lnb", [1, H])
    inp("mh", [2 * NV, T], BF16)   # multi-hot small-table encoding, stacked x2
    inp("oh", [128, T], BF16)      # one-hot slot->target-column scatter matrix
    inp("hl", [128, 1])            # header_len per slot (selected by col_idx)
    inp("widx", [128, NBLK], I32)  # word row per (p, j) token
    inp("wtidx", [128, 1], I32)    # word row at each slot's target token
    inp("hidx", [128, T // 16], I16)  # header gather rows, wrap16 layout
    inp("widx16", [128, T // 16], I16)   # word rows, wrap16 (dma_gather path)
    inp("wtidx16", [128, NSLOT // 16], I16)  # target word rows, wrap16
    inp("eye", [128, 128], BF16)         # identity for the pos matmul
    inp("poshi", [128, BPC * H], BF16)   # pos rows (p,j)-layout, hi part
    inp("poslo", [128, BPC * H], BF16)   # pos rows (p,j)-layout, lo part
    out = nc.dram_tensor("out", [BPC, S, H], F32, kind="ExternalOutput").ap()

    with tile.TileContext(nc) as tc, ExitStack() as ctx:
        _body(ctx, tc, t, out, skip_affine, mybir)
    nc.compile()
    return nc


def _body(ctx, tc, t, out, skip_affine, mybir):
    import concourse.bass as bass

    nc = tc.nc
    F32 = mybir.dt.float32
    BF16 = mybir.dt.bfloat16
    I16 = mybir.dt.int16
    I32 = mybir.dt.int32
    MUL = mybir.AluOpType.mult
    ADD = mybir.AluOpType.add
    AF = mybir.ActivationFunctionType

    const = ctx.enter_context(tc.tile_pool(name="const", bufs=1))
    setup = ctx.enter_context(tc.tile_pool(name="setup", bufs=1))
    hpool = ctx.enter_context(tc.tile_pool(name="hdr", bufs=3))
    h2pool = ctx.enter_context(tc.tile_pool(name="h2", bufs=2))
    h1pool = ctx.enter_context(tc.tile_pool(name="h1", bufs=2))
    wpool = ctx.enter_context(tc.tile_pool(name="wrd", bufs=3))
    epool = ctx.enter_context(tc.tile_pool(name="emb", bufs=3))
    opool = ctx.enter_context(tc.tile_pool(name="outp", bufs=3))
    qpool = ctx.enter_context(tc.tile_pool(name="sq", bufs=2))
    spool = ctx.enter_context(tc.tile_pool(name="stat", bufs=8))
    psum = ctx.enter_context(tc.tile_pool(name="ps", bufs=3, space="PSUM"))

    # ---------------- index tiles first (they gate the gathers) ----------
    s_hidx = const.tile([128, T // 16], I16)
    nc.sync.dma_start(s_hidx[:], t["hidx"])
    s_widx16_early = None
    if OPT_PE_POS or not OPT_INDIRECT:
        s_widx16_early = const.tile([128, T // 16], I16)
        nc.sync.dma_start(s_widx16_early[:], t["widx16"])
    s_wtidx = const.tile([128, 1], I32)
    nc.sync.dma_start(s_wtidx[:], t["wtidx"])
    s_widx = const.tile([128, NBLK], I32)
    nc.sync.dma_start(s_widx[:], t["widx"])

    # ---------------- remaining constants ----------------
    s_small = const.tile([2 * NV, H], BF16)
    nc.sync.dma_start(s_small[:], t["small3"])
    s_mh = const.tile([2 * NV, T], BF16)
    nc.sync.dma_start(s_mh[:], t["mh"])
    s_oh = const.tile([128, T], BF16)
    nc.sync.dma_start(s_oh[:], t["oh"])

    if OPT_PE_POS:
        s_eye = const.tile([128, 128], BF16)
        nc.sync.dma_start(s_eye[:], t["eye"])
        s_poshi = const.tile([128, BPC, H], BF16)
        nc.sync.dma_start(
            s_poshi[:], t["poshi"].rearrange("p (j h) -> p j h", h=H)
        )
        s_poslo = const.tile([128, BPC, H], BF16)
        nc.sync.dma_start(
            s_poslo[:], t["poslo"].rearrange("p (j h) -> p j h", h=H)
        )
        s_pos = None
    else:
        s_pos = const.tile([128, BPC, H], F32)
        nc.sync.dma_start(
            s_pos[:], t["pos_emb"].rearrange("(j p) h -> p j h", p=128)
        )

    s_eps = const.tile([128, 1], F32)
    nc.vector.memset(s_eps[:], EPS)

    if not skip_affine:
        s_lnw = const.tile([128, H], F32)
        nc.gpsimd.dma_start(s_lnw[:], t["lnw"].partition_broadcast(128))
        s_lnb = const.tile([128, H], F32)
        nc.gpsimd.dma_start(s_lnb[:], t["lnb"].partition_broadcast(128))

    s_hl = const.tile([128, 1], F32)
    nc.sync.dma_start(s_hl[:], t["hl"])
    s_recip = const.tile([128, 1], F32)
    nc.vector.tensor_scalar_max(s_recip[:], s_hl[:], 1.0)
    nc.vector.reciprocal(s_recip[:], s_recip[:])

    # ---------------- header gather + pooling ----------------
    # gather order i2 = l*128 + slot  ->  hch[:, m, :] holds l = 4*lc + m
    hacc = setup.tile([128, H], F32)
    hdr_gathers = []
    for lc in range(4):
        hch = hpool.tile([128, 4, H], F32)
        hdr_gathers.append(
            nc.gpsimd.dma_gather(
                hch[:], t["word_aug"], s_hidx[:, 32 * lc : 32 * (lc + 1)],
                512, 512, H,
            )
        )
        h2 = h2pool.tile([128, 2, H], F32)
        nc.vector.tensor_add(h2[:], hch[:, 0:2, :], hch[:, 2:4, :])
        if lc == 0:
            nc.vector.tensor_add(hacc[:], h2[:, 0, :], h2[:, 1, :])
        else:
            h1 = h1pool.tile([128, H], F32)
            nc.vector.tensor_add(h1[:], h2[:, 0, :], h2[:, 1, :])
            nc.vector.tensor_add(hacc[:], hacc[:], h1[:])

    # word rows at the scatter target tokens (for replacement deltas)
    wtgt = setup.tile([128, H], F32)
    if OPT_INDIRECT:
        nc.gpsimd.indirect_dma_start(
            wtgt[:], None, t["word_aug"],
            bass.IndirectOffsetOnAxis(ap=s_wtidx[:, 0:1], axis=0),
        )
    else:
        s_wtidx16 = const.tile([128, NSLOT // 16], I16)
        nc.sync.dma_start(s_wtidx16[:], t["wtidx16"])
        wtgt3 = wtgt[:].rearrange("p (o h) -> p o h", o=1)
        nc.gpsimd.dma_gather(
            wtgt3, t["word_aug"], s_wtidx16[:], NSLOT, NSLOT, H
        )
    # hadj = pooled - word_at_target (bf16 rhs for the scatter matmul);
    # invalid slots contribute nothing (their one-hot column is all zero)
    d = setup.tile([128, H], F32)
    nc.vector.tensor_scalar_mul(d[:], hacc[:], s_recip[:])
    nc.vector.tensor_sub(d[:], d[:], wtgt[:])
    hadj_hi = setup.tile([128, H], BF16)
    nc.vector.tensor_copy(hadj_hi[:], d[:])
    hadj_lo = setup.tile([128, H], BF16)
    nc.vector.tensor_sub(hadj_lo[:], d[:], hadj_hi[:])

    from concourse.tile import add_dep_helper

    s_widx16 = s_widx16_early

    # ---------------- token blocks ----------------
    inv_h = 1.0 / H
    for ch in range(BPC):
        wch = wpool.tile([128, 4, H], F32)
        if OPT_PE_POS:
            wg = nc.gpsimd.dma_gather(
                wch[:], t["word_aug"], s_widx16[:, 32 * ch : 32 * (ch + 1)],
                512, 512, H,
            )
            # keep the header gathers ahead of the bulk word gathers on the
            # SWDGE queue: the pooled slot rows gate every block's matmuls
            add_dep_helper(
                wg.ins, hdr_gathers[-1].ins, sync=False,
                reason="headers first on SWDGE queue",
            )
        elif OPT_ACC_GATHER:
            # prefill with positional rows, then accumulate gathered words
            # (one [P,1]-indexed indirect gather per 128-token block)
            nc.sync.dma_start(wch[:], s_pos[:])
            for jj in range(4):
                nc.gpsimd.indirect_dma_start(
                    wch[:, jj, :], None, t["word_aug"],
                    bass.IndirectOffsetOnAxis(
                        ap=s_widx[:, 4 * ch + jj : 4 * ch + jj + 1], axis=0
                    ),
                    compute_op=ADD,
                )
        elif OPT_INDIRECT:
            nc.gpsimd.indirect_dma_start(
                wch[:], None, t["word_aug"],
                bass.IndirectOffsetOnAxis(
                    ap=s_widx[:, 4 * ch : 4 * (ch + 1)], axis=0
                ),
            )
        else:
            nc.gpsimd.dma_gather(
                wch[:], t["word_aug"], s_widx16[:, 32 * ch : 32 * (ch + 1)],
                512, 512, H,
            )
        for jj in range(4):
            j = ch * 4 + jj
            ps = psum.tile([128, H], F32)
            lhs_mh = s_mh[:, j * 128 : (j + 1) * 128]
            lhs_oh = s_oh[:, j * 128 : (j + 1) * 128]
            for lo, hi in ((0, 512), (512, H)):
                nc.tensor.matmul(
                    ps[:, lo:hi], lhs_mh, s_small[:, lo:hi],
                    start=True, stop=False,
                )
                nc.tensor.matmul(
                    ps[:, lo:hi], lhs_oh, hadj_hi[:, lo:hi],
                    start=False, stop=False,
                )
                nc.tensor.matmul(
                    ps[:, lo:hi], lhs_oh, hadj_lo[:, lo:hi],
                    start=False, stop=not OPT_PE_POS,
                )
                if OPT_PE_POS:
                    nc.tensor.matmul(
                        ps[:, lo:hi], s_eye[:], s_poshi[:, jj, lo:hi],
                        start=False, stop=False,
                    )
                    nc.tensor.matmul(
                        ps[:, lo:hi], s_eye[:], s_poslo[:, jj, lo:hi],
                        start=False, stop=True,
                    )

            if OPT_PE_POS or OPT_ACC_GATHER:
                base = wch[:, jj, :]
            else:
                basetile = epool.tile([128, H], F32, tag="base")
                nc.vector.tensor_add(basetile[:], wch[:, jj, :], s_pos[:, jj, :])
                base = basetile[:]

            emb = epool.tile([128, H], F32)
            rsum = spool.tile([128, 1], F32)
            if OPT_TTR:
                # emb = (words+pos) + ps, and the row-sum, in one DVE pass
                nc.vector.tensor_tensor_reduce(
                    emb[:], base, ps[:], 1.0, 0.0, ADD, ADD, rsum[:]
                )
            else:
                nc.vector.tensor_add(emb[:], base, ps[:])

            u = spool.tile([128, 1], F32)
            var = spool.tile([128, 1], F32)
            if OPT_ACT_STATS:
                if not OPT_TTR:
                    # row sum on ACT (output write is a throwaway)
                    cp = qpool.tile([128, H], F32, tag="cp")
                    nc.scalar.activation(
                        cp[:], emb[:], AF.Copy, accum_out=rsum[:]
                    )
                # sum of squares on ACT
                sq = qpool.tile([128, H], F32)
                rsumsq = spool.tile([128, 1], F32)
                nc.scalar.activation(
                    sq[:], emb[:], AF.Square, accum_out=rsumsq[:]
                )
                # u, var  (tiny per-partition ops)
                nc.vector.tensor_scalar_mul(u[:], rsum[:], inv_h)
                mu2 = spool.tile([128, 1], F32)
                nc.vector.tensor_scalar(
                    mu2[:], u[:], u[:], -1.0, op0=MUL, op1=MUL
                )
                nc.vector.tensor_scalar(
                    var[:], rsumsq[:], inv_h, mu2[:], op0=MUL, op1=ADD
                )
            else:
                stats = spool.tile([128, 2, 6], F32)
                for g in range(2):
                    nc.vector.bn_stats(
                        stats[:, g, :], emb[:, g * 384 : (g + 1) * 384]
                    )
                mv = spool.tile([128, 2], F32)
                nc.vector.bn_aggr(mv[:], stats[:])
                nc.vector.tensor_copy(u[:], mv[:, 0:1])
                nc.vector.tensor_copy(var[:], mv[:, 1:2])

            rstd = spool.tile([128, 1], F32)
            nc.scalar.activation(
                rstd[:], var[:], AF.Sqrt, bias=s_eps[:], scale=1.0
            )
            nc.vector.reciprocal(rstd[:], rstd[:])

            o = opool.tile([128, H], F32)
            if OPT_ACT_APPLY:
                nub = spool.tile([128, 1], F32)
                nc.vector.tensor_scalar(
                    nub[:], u[:], rstd[:], -1.0, op0=MUL, op1=MUL
                )
                # LN apply on ACT: out = rstd*emb - u*rstd
                nc.scalar.activation(
                    o[:], emb[:], AF.Identity, bias=nub[:], scale=rstd[:]
                )
            else:
                nc.vector.tensor_scalar(
                    o[:], emb[:], u[:], rstd[:],
                    op0=mybir.AluOpType.subtract, op1=MUL,
                )
            if not skip_affine:
                nc.vector.tensor_mul(o[:], o[:], s_lnw[:])
                nc.vector.tensor_add(o[:], o[:], s_lnb[:])

            nc.sync.dma_start(out[ch, jj * 128 : (jj + 1) * 128, :], o[:])


def _wrap16(flat):
    w = flat.reshape(-1, 16).T.astype(np.int16)
    return np.tile(w, (8, 1))


def _prep_core(core, iid, hdr, tt, mt, ti, cpos, cidx, hlen):
    import ml_dtypes

    b0 = core * BPC
    sl = slice(b0, b0 + BPC)
    iids = iid[sl]

    # word gather rows in (p, ch*4+j) layout for the indirect gathers
    widx = np.ascontiguousarray(
        iids.reshape(BPC * 4, 128).T.astype(np.int32)
    )  # [128, NBLK] ; widx[p, j] = token j*128+p

    bb = np.arange(BPC)[:, None]
    sel_hdr = hdr[sl][bb, cidx[sl]]                      # [BPC, C, L]
    sel_len = hlen[sl][bb, cidx[sl]]                     # [BPC, C]
    maskl = np.arange(L)[None, None, :] < sel_len[:, :, None]
    hvals = np.where(maskl, sel_hdr, ZROW)               # [BPC, C, L]
    hflat = hvals.reshape(NSLOT, L).T.reshape(-1)        # i2 = l*128 + slot
    hidx = _wrap16(hflat)

    # word rows at each slot's target position
    wtidx = iids[bb, cpos[sl]].reshape(NSLOT, 1).astype(np.int32)

    # one-hot scatter matrix [128 slots, T] (bf16): column = local target token
    tgt = np.where(
        sel_len.reshape(-1) > 0, (bb * S + cpos[sl]).reshape(-1), -1
    )
    oh = np.zeros((NSLOT, T), dtype=ml_dtypes.bfloat16)
    valid = tgt >= 0
    oh[np.arange(NSLOT)[valid], tgt[valid]] = 1

    # multi-hot small-table encoding [19, T], stacked twice for the
    # hi/lo error-compensated small-table matmul
    mh1 = np.zeros((NV, T), dtype=ml_dtypes.bfloat16)
    ar = np.arange(T)
    mh1[tt[sl].reshape(-1), ar] = 1
    mh1[2 + mt[sl].reshape(-1), ar] += 1
    mh1[13 + ti[sl].reshape(-1), ar] += 1
    mh = np.concatenate([mh1, mh1], axis=0)

    hl = sel_len.reshape(NSLOT, 1).astype(np.float32)
    widx16 = _wrap16(iids.reshape(-1))
    wtidx16 = _wrap16(wtidx.reshape(-1))
    return widx, wtidx, hidx, oh, mh, hl, widx16, wtidx16


def make_in_maps(inputs):
    import ml_dtypes

    inp = {k: np.asarray(v) for k, v in inputs.items()}
    word = np.ascontiguousarray(inp["word_emb"], dtype=np.float32)
    word_aug = np.concatenate([word, np.zeros((1, H), np.float32)], axis=0)
    small3_f32 = np.concatenate(
        [inp["tok_type_emb"], inp["match_emb"], inp["type_emb"]], axis=0
    ).astype(np.float32)
    small_hi = small3_f32.astype(ml_dtypes.bfloat16)
    small_lo = (small3_f32 - small_hi.astype(np.float32)).astype(
        ml_dtypes.bfloat16
    )
    small3 = np.concatenate([small_hi, small_lo], axis=0)
    pos = np.ascontiguousarray(inp["pos_emb"], dtype=np.float32)
    lnw = np.ascontiguousarray(inp["ln_w"], dtype=np.float32).reshape(1, H)
    lnb = np.ascontiguousarray(inp["ln_b"], dtype=np.float32).reshape(1, H)
    skip_affine = bool(np.all(lnw == 1.0) and np.all(lnb == 0.0))

    iid = inp["input_ids"].astype(np.int64)
    hdr = inp["header_ids"].astype(np.int64)
    tt = inp["token_type_ids"].astype(np.int64)
    mt = inp["match_type_ids"].astype(np.int64)
    ti = inp["type_idx"].astype(np.int64)
    cpos = inp["col_pos"].astype(np.int64)
    cidx = inp["col_idx"].astype(np.int64)
    hlen = inp["header_len"].astype(np.int64)

    # pos rows in (p, j)-block layout, split hi/lo for exact bf16 matmul adds
    posp = np.ascontiguousarray(
        pos.reshape(4, 128, H).transpose(1, 0, 2).reshape(128, 4 * H)
    )
    poshi = posp.astype(ml_dtypes.bfloat16)
    poslo = (posp - poshi.astype(np.float32)).astype(ml_dtypes.bfloat16)
    eye = np.eye(128, dtype=ml_dtypes.bfloat16)

    in_maps = []
    for core in range(NCORES):
        widx, wtidx, hidx, oh, mh, hl, widx16, wtidx16 = _prep_core(
            core, iid, hdr, tt, mt, ti, cpos, cidx, hlen
        )
        m = dict(
            word_aug=word_aug, pos_emb=pos, small3=small3,
            mh=mh, oh=oh, hl=hl, widx=widx, wtidx=wtidx, hidx=hidx,
            widx16=widx16, wtidx16=wtidx16,
            eye=eye, poshi=poshi, poslo=poslo,
        )
        if not skip_affine:
            m["lnw"] = lnw
            m["lnb"] = lnb
        in_maps.append(m)
    return in_maps, skip_affine


def get_nc(skip_affine):
    if skip_affine not in _NC_CACHE:
        _NC_CACHE[skip_affine] = _build_nc(skip_affine)
    return _NC_CACHE[skip_affine]


def run_hw(inputs, trace=False, trace_cores=None):
    """Returns (out [B,S,H] f32, BassKernelResults)."""
    from concourse.bass_utils import run_bass_kernel_spmd

    in_maps, skip_affine = make_in_maps(inputs)
    nc = get_nc(skip_affine)
    res = run_bass_kernel_spmd(
        nc, in_maps, core_ids=list(range(NCORES)), trace=trace,
        trace_cores=trace_cores,
    )
    out = np.concatenate([res.results[c]["out"] for c in range(NCORES)], axis=0)
    return out, res


def kernel(**inputs) -> np.ndarray:
    out, _ = run_hw(inputs, trace=False)
    return out


# revision 36
# speedup vs baseline: 1.1354x; 1.0121x over previous
"""BertEmbeddings (word lookup + header mean-pool scatter + pos/type/match
embeddings + TF-style LayerNorm) as a Bass/Tile kernel on 8 trn2 NeuronCores.

Sharding: data-parallel over batch (4 rows/core); embedding tables replicated.

Per-core device pipeline (v4):
  - 16 main 128-token blocks run WITHOUT any header dependency:
      words via dma_gather; pos + small-table adds via bf16 matmuls into one
      PSUM accumulation whose extra 769th column carries every operand's
      row-sum (word row-sums ride the multihot as two bf16 value rows), so
      the LN mean is free; sum-of-squares via one ACT Square pass
      (accum_out); LN apply on ACT as Identity(scale=rstd, bias=-u*rstd)
  - the scattered columns are handled by ONE extra "target block" at the
    end: header rows gathered (slot order, zero-row padding), mean-pooled,
    combined with gathered pos rows and the targets' small-table one-hots,
    LayerNormed, then written over the affected tokens with a bounds-checked
    indirect scatter-store (invalid slots point out of bounds -> skipped)

All data-dependent arithmetic on embedding VALUES runs on device; the host
only reformats index tensors and precomputes input-independent table
derivatives (zero-row padding, row sums, one-hot/multi-hot encodings).
"""

import numpy as np

B, S, H = 32, 512, 768
VOCAB = 30522
NCORES = 8
BPC = B // NCORES            # batch rows per core
T = BPC * S                  # tokens per core
NBLK = T // 128              # 128-token blocks per core
C, L = 32, 16                # columns, max header len
NSLOT = BPC * C              # 128 slots per core
ZROW = VOCAB                 # zero row in augmented word table
WROWS = VOCAB + 1
NV = 19                      # 2 + 11 + 6 small-table rows
KMH = 2 * NV + 2             # multihot rows: [hi; lo; wsum_hi; wsum_lo]
HE = H + 1                   # embedding columns + row-sum column
EPS = 1e-12

_NC_CACHE = {}

import os as _os

OPT_F16 = _os.environ.get("K_F16", "0") == "1"  # fp16 word table (halves gathers)


def _build_nc(skip_affine: bool):
    from contextlib import ExitStack

    import concourse.bacc as bacc
    import concourse.tile as tile
    from concourse import mybir

    BF16 = mybir.dt.bfloat16
    I16 = mybir.dt.int16
    I32 = mybir.dt.int32
    F32 = mybir.dt.float32
    WDT = mybir.dt.float16 if OPT_F16 else F32

    nc = bacc.Bacc("TRN2", target_bir_lowering=False, debug=False)
    t = {}

    def inp(name, shape, dt=F32):
        t[name] = nc.dram_tensor(name, shape, dt, kind="ExternalInput").ap()

    inp("word_aug", [WROWS, H], WDT)
    inp("small2", [KMH, HE], BF16)   # [hi;lo;wsum-slot] tables + row-sum col
    inp("mh", [KMH, T], BF16)        # multihot (x2) + word row-sum value rows
    inp("poshi", [128, BPC * HE], BF16)  # pos rows (p,j)-layout + sum col, hi
    inp("poslo", [128, BPC * HE], BF16)  # lo part
    inp("eye", [128, 128], BF16)
    inp("pos_aug", [S, HE])          # f32 pos rows + row sum (target block)
    inp("hl", [128, 1])              # header_len per slot
    inp("hidx", [128, T // 16], I16)     # header gather rows, wrap16
    inp("widx16", [128, T // 16], I16)   # word gather rows, wrap16
    inp("wtidx", [128, 1], I32)      # word row at each slot's target token
    inp("posidx", [128, 1], I32)     # pos row for each slot's target token
    inp("mht", [KMH, 128], BF16)     # multihot for target tokens (no wsum)
    inp("tgtrow", [128, 1], I32)     # out row per slot (OOB for invalid)
    if not skip_affine:
        inp("lnw", [1, H])
        inp("lnb", [1, H])
    out = nc.dram_tensor("out", [BPC, S, H], F32, kind="ExternalOutput").ap()

    with tile.TileContext(nc) as tc, ExitStack() as ctx:
        _body(ctx, tc, t, out, skip_affine, mybir)
    nc.compile()
    return nc


def _body(ctx, tc, t, out, skip_affine, mybir):
    import concourse.bass as bass
    from concourse.tile import add_dep_helper

    nc = tc.nc
    F32 = mybir.dt.float32
    BF16 = mybir.dt.bfloat16
    I16 = mybir.dt.int16
    I32 = mybir.dt.int32
    WDT = mybir.dt.float16 if OPT_F16 else F32
    MUL = mybir.AluOpType.mult
    ADD = mybir.AluOpType.add
    AF = mybir.ActivationFunctionType

    const = ctx.enter_context(tc.tile_pool(name="const", bufs=1))
    setup = ctx.enter_context(tc.tile_pool(name="setup", bufs=1))
    hpool = ctx.enter_context(tc.tile_pool(name="hdr", bufs=3))
    h2pool = ctx.enter_context(tc.tile_pool(name="h2", bufs=2))
    h1pool = ctx.enter_context(tc.tile_pool(name="h1", bufs=2))
    wpool = ctx.enter_context(tc.tile_pool(name="wrd", bufs=4))
    epool = ctx.enter_context(tc.tile_pool(name="emb", bufs=3))
    opool = ctx.enter_context(tc.tile_pool(name="outp", bufs=3))
    qpool = ctx.enter_context(tc.tile_pool(name="sq", bufs=2))
    spool = ctx.enter_context(tc.tile_pool(name="stat", bufs=8))
    psum = ctx.enter_context(tc.tile_pool(name="ps", bufs=3, space="PSUM"))
    psumt = ctx.enter_context(tc.tile_pool(name="pst", bufs=1, space="PSUM"))

    # ---------------- index tiles first (they gate the gathers) ----------
    s_widx16 = const.tile([128, T // 16], I16)
    nc.sync.dma_start(s_widx16[:], t["widx16"])
    s_hidx = const.tile([128, T // 16], I16)
    nc.sync.dma_start(s_hidx[:], t["hidx"])
    s_wtidx = const.tile([128, 1], I32)
    nc.sync.dma_start(s_wtidx[:], t["wtidx"])
    s_posidx = const.tile([128, 1], I32)
    nc.sync.dma_start(s_posidx[:], t["posidx"])
    s_tgtrow = const.tile([128, 1], I32)
    nc.sync.dma_start(s_tgtrow[:], t["tgtrow"])

    # ---------------- word gathers (front of the SWDGE queue) ------------
    wchs = []
    word_gathers = []
    for ch in range(BPC):
        wch = wpool.tile([128, 4, H], WDT)
        word_gathers.append(
            nc.gpsimd.dma_gather(
                wch[:], t["word_aug"], s_widx16[:, 32 * ch : 32 * (ch + 1)],
                512, 512, H,
            )
        )
        wchs.append(wch)

    # ---------------- remaining constants ----------------
    s_small = const.tile([KMH, HE], BF16)
    nc.sync.dma_start(s_small[:], t["small2"])
    s_mh = const.tile([KMH, T], BF16)
    nc.sync.dma_start(s_mh[:], t["mh"])
    s_mht = const.tile([KMH, 128], BF16)
    nc.sync.dma_start(s_mht[:], t["mht"])
    s_eye = const.tile([128, 128], BF16)
    nc.sync.dma_start(s_eye[:], t["eye"])
    s_poshi = const.tile([128, BPC, HE], BF16)
    nc.sync.dma_start(s_poshi[:], t["poshi"].rearrange("p (j h) -> p j h", h=HE))
    s_poslo = const.tile([128, BPC, HE], BF16)
    nc.sync.dma_start(s_poslo[:], t["poslo"].rearrange("p (j h) -> p j h", h=HE))

    s_eps = const.tile([128, 1], F32)
    nc.vector.memset(s_eps[:], EPS)

    if not skip_affine:
        s_lnw = const.tile([128, H], F32)
        nc.gpsimd.dma_start(s_lnw[:], t["lnw"].partition_broadcast(128))
        s_lnb = const.tile([128, H], F32)
        nc.gpsimd.dma_start(s_lnb[:], t["lnb"].partition_broadcast(128))

    s_hl = const.tile([128, 1], F32)
    nc.sync.dma_start(s_hl[:], t["hl"])
    s_recip = const.tile([128, 1], F32)
    nc.vector.tensor_scalar_max(s_recip[:], s_hl[:], 1.0)
    nc.vector.reciprocal(s_recip[:], s_recip[:])

    # ---------------- header gathers (behind the word gathers) ----------
    hacc = setup.tile([128, H], F32)
    hdr_gathers = []
    for lc in range(4):
        hch = hpool.tile([128, 4, H], WDT)
        g = nc.gpsimd.dma_gather(
            hch[:], t["word_aug"], s_hidx[:, 32 * lc : 32 * (lc + 1)],
            512, 512, H,
        )
        add_dep_helper(
            g.ins, word_gathers[-1].ins, sync=False,
            reason="word gathers first on SWDGE queue",
        )
        hdr_gathers.append(g)
        h2 = h2pool.tile([128, 2, H], F32)
        nc.vector.tensor_add(h2[:], hch[:, 0:2, :], hch[:, 2:4, :])
        if lc == 0:
            nc.vector.tensor_add(hacc[:], h2[:, 0, :], h2[:, 1, :])
        else:
            h1 = h1pool.tile([128, H], F32)
            nc.vector.tensor_add(h1[:], h2[:, 0, :], h2[:, 1, :])
            nc.vector.tensor_add(hacc[:], hacc[:], h1[:])

    # word + f32 pos rows at the scatter target tokens
    wtgt = setup.tile([128, H], WDT)
    g = nc.gpsimd.indirect_dma_start(
        wtgt[:], None, t["word_aug"],
        bass.IndirectOffsetOnAxis(ap=s_wtidx[:, 0:1], axis=0),
    )
    add_dep_helper(g.ins, hdr_gathers[-1].ins, sync=False, reason="after hdrs")
    postgt = setup.tile([128, HE], F32)
    nc.gpsimd.indirect_dma_start(
        postgt[:], None, t["pos_aug"],
        bass.IndirectOffsetOnAxis(ap=s_posidx[:, 0:1], axis=0),
    )

    # pooled slot rows (mean over valid header tokens; invalid slots -> 0)
    pooled = setup.tile([128, H], F32)
    nc.vector.tensor_scalar_mul(pooled[:], hacc[:], s_recip[:])

    # ---------------- main token blocks (no header dependency) -----------
    inv_h = 1.0 / H
    stores = []
    for ch in range(BPC):
        wch = wchs[ch]
        for jj in range(4):
            j = ch * 4 + jj
            ps = psum.tile([128, HE], F32)
            lhs_mh = s_mh[:, j * 128 : (j + 1) * 128]
            for lo, hi in ((0, 512), (512, HE)):
                nc.tensor.matmul(
                    ps[:, lo:hi], lhs_mh, s_small[:, lo:hi],
                    start=True, stop=False,
                )
                nc.tensor.matmul(
                    ps[:, lo:hi], s_eye[:], s_poshi[:, jj, lo:hi],
                    start=False, stop=False,
                )
                nc.tensor.matmul(
                    ps[:, lo:hi], s_eye[:], s_poslo[:, jj, lo:hi],
                    start=False, stop=True,
                )

            # emb = words + ps ; row-sum comes from ps column H
            emb = epool.tile([128, H], F32)
            nc.vector.tensor_add(emb[:], wch[:, jj, :], ps[:, 0:H])

            sq = qpool.tile([128, H], F32)
            rsumsq = spool.tile([128, 1], F32)
            nc.scalar.activation(sq[:], emb[:], AF.Square, accum_out=rsumsq[:])

            u = spool.tile([128, 1], F32)
            nc.vector.tensor_scalar_mul(u[:], ps[:, H : H + 1], inv_h)
            mu2 = spool.tile([128, 1], F32)
            nc.vector.tensor_scalar(mu2[:], u[:], u[:], -1.0, op0=MUL, op1=MUL)
            var = spool.tile([128, 1], F32)
            nc.vector.tensor_scalar(
                var[:], rsumsq[:], inv_h, mu2[:], op0=MUL, op1=ADD
            )
            rstd = spool.tile([128, 1], F32)
            nc.scalar.activation(
                rstd[:], var[:], AF.Sqrt, bias=s_eps[:], scale=1.0
            )
            nc.vector.reciprocal(rstd[:], rstd[:])
            nub = spool.tile([128, 1], F32)
            nc.vector.tensor_scalar(nub[:], u[:], rstd[:], -1.0, op0=MUL, op1=MUL)

            o = opool.tile([128, H], F32)
            nc.scalar.activation(
                o[:], emb[:], AF.Identity, bias=nub[:], scale=rstd[:]
            )
            if not skip_affine:
                nc.vector.tensor_mul(o[:], o[:], s_lnw[:])
                nc.vector.tensor_add(o[:], o[:], s_lnb[:])

            stores.append(
                nc.sync.dma_start(out[ch, jj * 128 : (jj + 1) * 128, :], o[:])
            )

    # ---------------- target block: scattered tokens ----------------
    ps_t = psumt.tile([128, HE], F32)
    for lo, hi in ((0, 512), (512, HE)):
        nc.tensor.matmul(
            ps_t[:, lo:hi], s_mht[:], s_small[:, lo:hi], start=True, stop=True
        )
    emb_t = setup.tile([128, H], F32)
    nc.vector.tensor_add(emb_t[:], pooled[:], postgt[:, 0:H])
    nc.vector.tensor_add(emb_t[:], emb_t[:], ps_t[:, 0:H])

    stats = spool.tile([128, 2, 6], F32)
    for g2 in range(2):
        nc.vector.bn_stats(stats[:, g2, :], emb_t[:, g2 * 384 : (g2 + 1) * 384])
    mv = spool.tile([128, 2], F32)
    nc.vector.bn_aggr(mv[:], stats[:])
    rstd_t = spool.tile([128, 1], F32)
    nc.scalar.activation(
        rstd_t[:], mv[:, 1:2], AF.Sqrt, bias=s_eps[:], scale=1.0
    )
    nc.vector.reciprocal(rstd_t[:], rstd_t[:])
    o_t = setup.tile([128, H], F32)
    nc.vector.tensor_scalar(
        o_t[:], emb_t[:], mv[:, 0:1], rstd_t[:],
        op0=mybir.AluOpType.subtract, op1=MUL,
    )
    if not skip_affine:
        nc.vector.tensor_mul(o_t[:], o_t[:], s_lnw[:])
        nc.vector.tensor_add(o_t[:], o_t[:], s_lnb[:])

    # overwrite the scattered rows; invalid slots point out of bounds and
    # are silently skipped via the bounds check
    scat = nc.gpsimd.indirect_dma_start(
        out.rearrange("b s h -> (b s) h"),
        bass.IndirectOffsetOnAxis(ap=s_tgtrow[:, 0:1], axis=0),
        o_t[:], None,
        bounds_check=T - 1, oob_is_err=False,
    )
    for st in stores:
        add_dep_helper(
            scat.ins, st.ins, sync=True, reason="scatter after block stores"
        )


def _wrap16(flat):
    w = flat.reshape(-1, 16).T.astype(np.int16)
    return np.tile(w, (8, 1))


def _multihot(tt, mt, ti, n, dtype):
    mh1 = np.zeros((NV, n), dtype=dtype)
    ar = np.arange(n)
    mh1[tt, ar] = 1
    mh1[2 + mt, ar] += 1
    mh1[13 + ti, ar] += 1
    return mh1


def _prep_core(core, iid, hdr, tt, mt, ti, cpos, cidx, hlen, wsum, bf16):
    b0 = core * BPC
    sl = slice(b0, b0 + BPC)
    iids = iid[sl]

    widx16 = _wrap16(iids.reshape(-1))

    bb = np.arange(BPC)[:, None]
    sel_hdr = hdr[sl][bb, cidx[sl]]                      # [BPC, C, L]
    sel_len = hlen[sl][bb, cidx[sl]]                     # [BPC, C]
    maskl = np.arange(L)[None, None, :] < sel_len[:, :, None]
    hvals = np.where(maskl, sel_hdr, ZROW)               # [BPC, C, L]
    hflat = hvals.reshape(NSLOT, L).T.reshape(-1)        # i2 = l*128 + slot
    hidx = _wrap16(hflat)

    wtidx = iids[bb, cpos[sl]].reshape(NSLOT, 1).astype(np.int32)
    posidx = cpos[sl].reshape(NSLOT, 1).astype(np.int32)
    hl = sel_len.reshape(NSLOT, 1).astype(np.float32)

    valid = sel_len.reshape(-1) > 0
    tgtrow = np.where(
        valid, (bb * S + cpos[sl]).reshape(-1), 10 * T
    ).astype(np.int32).reshape(NSLOT, 1)

    # multihot (x2 for hi/lo) + word row-sum value rows
    ttf, mtf, tif = tt[sl].reshape(-1), mt[sl].reshape(-1), ti[sl].reshape(-1)
    mh1 = _multihot(ttf, mtf, tif, T, bf16)
    ws = wsum[iids.reshape(-1)].astype(np.float32)
    ws_hi = ws.astype(bf16)
    ws_lo = (ws - ws_hi.astype(np.float32)).astype(bf16)
    mh = np.concatenate([mh1, mh1, ws_hi[None, :], ws_lo[None, :]], axis=0)

    # target tokens' multihot (no word rows; invalid slots -> token ids 0)
    s_t = cpos[sl].reshape(-1)
    tt_t = tt[sl][bb, cpos[sl]].reshape(-1)
    mt_t = mt[sl][bb, cpos[sl]].reshape(-1)
    ti_t = ti[sl][bb, cpos[sl]].reshape(-1)
    mh1t = _multihot(tt_t, mt_t, ti_t, NSLOT, bf16)
    mht = np.concatenate(
        [mh1t, mh1t, np.zeros((2, NSLOT), dtype=bf16)], axis=0
    )
    return widx16, hidx, wtidx, posidx, hl, tgtrow, mh, mht


def _hi_lo_with_sums(mat, bf16):
    """Split [N, H] f32 into bf16 hi/lo parts, extended with a row-sum
    column: the TOTAL row sum of the stored values is itself hi/lo encoded
    across the two parts' sum columns (so u stays ~f32-accurate)."""
    hi = mat.astype(bf16)
    lo = (mat - hi.astype(np.float32)).astype(bf16)
    total = (
        hi.astype(np.float64).sum(1) + lo.astype(np.float64).sum(1)
    ).astype(np.float32)
    s_hi = total.astype(bf16)
    s_lo = (total - s_hi.astype(np.float32)).astype(bf16)
    hie = np.concatenate([hi, s_hi[:, None]], axis=1)
    loe = np.concatenate([lo, s_lo[:, None]], axis=1)
    return hie, loe


def make_in_maps(inputs):
    import ml_dtypes

    bf16 = ml_dtypes.bfloat16
    inp = {k: np.asarray(v) for k, v in inputs.items()}
    word = np.ascontiguousarray(inp["word_emb"], dtype=np.float32)
    if OPT_F16:
        word_aug = np.concatenate(
            [word.astype(np.float16), np.zeros((1, H), np.float16)], axis=0
        )
    else:
        word_aug = np.concatenate([word, np.zeros((1, H), np.float32)], axis=0)
    # row sums of the table as stored on device (feeds the LN mean)
    wsum = word_aug.astype(np.float64).sum(1).astype(np.float32)

    small3_f32 = np.concatenate(
        [inp["tok_type_emb"], inp["match_emb"], inp["type_emb"]], axis=0
    ).astype(np.float32)
    s_hie, s_loe = _hi_lo_with_sums(small3_f32, bf16)
    # wsum value rows contribute only to the row-sum column
    wrows = np.zeros((2, HE), dtype=bf16)
    wrows[:, H] = 1
    small2 = np.concatenate([s_hie, s_loe, wrows], axis=0)

    pos = np.ascontiguousarray(inp["pos_emb"], dtype=np.float32)
    p_hie, p_loe = _hi_lo_with_sums(pos, bf16)  # [512, HE] each
    poshi = np.ascontiguousarray(
        p_hie.reshape(4, 128, HE).transpose(1, 0, 2).reshape(128, 4 * HE)
    )
    poslo = np.ascontiguousarray(
        p_loe.reshape(4, 128, HE).transpose(1, 0, 2).reshape(128, 4 * HE)
    )
    pos_aug = np.concatenate(
        [pos, pos.astype(np.float64).sum(1).astype(np.float32)[:, None]], axis=1
    )
    eye = np.eye(128, dtype=bf16)

    lnw = np.ascontiguousarray(inp["ln_w"], dtype=np.float32).reshape(1, H)
    lnb = np.ascontiguousarray(inp["ln_b"], dtype=np.float32).reshape(1, H)
    skip_affine = bool(np.all(lnw == 1.0) and np.all(lnb == 0.0))

    iid = inp["input_ids"].astype(np.int64)
    hdr = inp["header_ids"].astype(np.int64)
    tt = inp["token_type_ids"].astype(np.int64)
    mt = inp["match_type_ids"].astype(np.int64)
    ti = inp["type_idx"].astype(np.int64)
    cpos = inp["col_pos"].astype(np.int64)
    cidx = inp["col_idx"].astype(np.int64)
    hlen = inp["header_len"].astype(np.int64)

    in_maps = []
    for core in range(NCORES):
        widx16, hidx, wtidx, posidx, hl, tgtrow, mh, mht = _prep_core(
            core, iid, hdr, tt, mt, ti, cpos, cidx, hlen, wsum, bf16
        )
        m = dict(
            word_aug=word_aug, small2=small2, mh=mh, mht=mht,
            poshi=poshi, poslo=poslo, eye=eye, pos_aug=pos_aug,
            hl=hl, hidx=hidx, widx16=widx16, wtidx=wtidx,
            posidx=posidx, tgtrow=tgtrow,
        )
        if not skip_affine:
            m["lnw"] = lnw
            m["lnb"] = lnb
        in_maps.append(m)
    return in_maps, skip_affine


def get_nc(skip_affine):
    if skip_affine not in _NC_CACHE:
        _NC_CACHE[skip_affine] = _build_nc(skip_affine)
    return _NC_CACHE[skip_affine]


def run_hw(inputs, trace=False, trace_cores=None):
    """Returns (out [B,S,H] f32, BassKernelResults)."""
    from concourse.bass_utils import run_bass_kernel_spmd

    in_maps, skip_affine = make_in_maps(inputs)
    nc = get_nc(skip_affine)
    res = run_bass_kernel_spmd(
        nc, in_maps, core_ids=list(range(NCORES)), trace=trace,
        trace_cores=trace_cores,
    )
    out = np.concatenate([res.results[c]["out"] for c in range(NCORES)], axis=0)
    return out, res


def kernel(**inputs) -> np.ndarray:
    out, _ = run_hw(inputs, trace=False)
    return out
